# revision 72
# baseline (speedup 1.0000x reference)
"""Causal multi-head attention block (B=2, T=2048, C=1024, H=16) on 8 TRN2 cores.

Sharding: tensor-parallel over heads x data-parallel over batch.
Core c handles batch b = c // 4 and head-group hg = c % 4 (4 heads = 256 of
the 1024 channel columns). Each core computes, for its batch and heads:
    QT/KT = (Wslice/8)^T X^T + b/8   (scores pre-scaled by 1/sqrt(D))
    V     = X Wv_slice + bv
    S^T   = K Q^T (causal, streamed in 128x512 chunks), P = exp(S^T)
    O^T   = [V|1]^T P  -> rows 0..63 unnormalized output, row 64 softmax sum
    partial = (O^T normalized)^T @ Wo_rows_slice        [2048, 1024]
Host sums the 4 partials per batch and adds bo.

MM_DT selects the PE datapath: float32r (full-rate fp32 path, ~1.6e-4 rel
per matmul) or float32 (exact, 4x slower rows).
"""

from contextlib import ExitStack

import numpy as np

import concourse.bacc as bacc
import concourse.mybir as mybir
import concourse.tile as tile
from concourse.bass_utils import run_bass_kernel_spmd

B, T, C, H, D = 2, 2048, 1024, 16, 64
N_CORES = 8
HG = 4                  # head-groups (tensor parallel)
HPC = H // HG           # heads per core = 4
HD = HPC * D            # channel slice per core = 256
P = 128                 # partitions
NT = T // 512           # 4 i-tiles of 512
NIC = T // P            # 16 i-chunks of 128
NKC = C // P            # 8 contraction chunks of 128
F32 = mybir.dt.float32
F32R = mybir.dt.float32r
AF = mybir.ActivationFunctionType

MM_DT = F32R            # matmul datapath dtype (F32R fast / F32 exact)

_CACHE: dict = {}


def _build_program():
    nc = bacc.Bacc("TRN2", debug=False)

    XT = nc.dram_tensor("XT", [C, T], MM_DT, kind="ExternalInput").ap()
    WQKV = nc.dram_tensor("WQKV", [C, 3 * HD], MM_DT, kind="ExternalInput").ap()
    BQK = nc.dram_tensor("BQK", [P, 4], F32, kind="ExternalInput").ap()
    BV = nc.dram_tensor("BV", [1, HD], MM_DT, kind="ExternalInput").ap()
    WO = nc.dram_tensor("WO", [HD, C], MM_DT, kind="ExternalInput").ap()
    OUT = nc.dram_tensor("OUT", [T, C], F32, kind="ExternalOutput").ap()

    # Causal chunk mask: for diagonal chunk k (k=0..3), valid iff f >= p + 128k,
    # realized as slices of Mbig[p, x] = (x >= p + 384).
    mb = (np.arange(512)[None, :] >= np.arange(P)[:, None]).astype(np.float32)
    MBIG = nc.inline_tensor(mb, name="mbig").ap()
    ONES = nc.inline_tensor(np.ones((1, P), np.float32), name="ones_c").ap()
    VONES = nc.inline_tensor(np.ones((P, NIC * HPC), np.float32), name="vones_c").ap()

    with tile.TileContext(nc) as tc:
        _trace_kernel(tc, XT, WQKV, BQK, BV, WO, OUT, MBIG, ONES, VONES)
    nc.compile()
    return nc


def _trace_kernel(tc, XT, WQKV, BQK, BV, WO, OUT, MBIG, ONES, VONES):
    nc = tc.nc

    with ExitStack() as ctx:
        consts = ctx.enter_context(tc.tile_pool(name="consts", bufs=1))
        wpool = ctx.enter_context(tc.tile_pool(name="weights", bufs=1))
        xpool = ctx.enter_context(tc.tile_pool(name="xt", bufs=1))
        qkv = ctx.enter_context(tc.tile_pool(name="qkv", bufs=1))

        # Two HWDGE queues: SP (nc.sync) and Activation (nc.scalar). Interleave
        # the big loads across both so stage A is compute- not DMA-paced.
        qs, qa = nc.sync, nc.scalar

        # ---- tiles ----
        mbig_sb = consts.tile([P, 512], MM_DT, name="mbig_sb")
        ones_sb = consts.tile([1, P], MM_DT, name="ones_sb")
        bias_sb = consts.tile([P, 4], F32, name="bias_sb")  # bq m0,m1, bk m0,m1
        bv_sb = consts.tile([1, HD], MM_DT, name="bv_sb")
        wqkv_sb = wpool.tile([P, NKC, 3 * HD], MM_DT, name="wqkv_sb")
        wo_sb = wpool.tile([P, 2, C], MM_DT, name="wo_sb")
        xts = [
            xpool.tile([P, T], MM_DT, name=f"xt{kc}", tag=f"xt{kc}")
            for kc in range(NKC)
        ]
        qt_sb = [qkv.tile([P, T], MM_DT, name=f"qt{m}", tag=f"qt{m}") for m in range(2)]
        kt_sb = [qkv.tile([P, T], MM_DT, name=f"kt{m}", tag=f"kt{m}") for m in range(2)]
        v_sb = qkv.tile([P, NIC, HPC, D + 1], MM_DT, name="v_sb")
        ot_sb = [qkv.tile([P, T], MM_DT, name=f"ot{m}", tag=f"ot{m}") for m in range(2)]

        def wq_c(kc, msl):
            return wqkv_sb[:, kc, msl]

        def wk_c(kc, msl):
            return wqkv_sb[:, kc, slice(HD + msl.start, HD + msl.stop)]

        def wv_c(kc):
            return wqkv_sb[:, kc, 2 * HD : 3 * HD]

        # Preload the ACT Exp table while the first DMAs stream (the table
        # load costs ~1.3us and would otherwise land on the first real exp).
        scx = consts.tile([1, 1], F32, name="scx")
        nc.vector.memset(scx, 0.0)
        scy = consts.tile([1, 1], F32, name="scy")
        nc.scalar.activation(scy, scx, AF.Exp)

        # ---- loads ----
        # Weights and the t=0 column-block of X^T stream first (kc-ordered,
        # alternating across both HWDGE queues) so t=0 projections complete
        # after ~5MB; remaining X^T column-blocks stream per i-tile behind
        # them and the whole pipeline becomes PE-paced after ~15us.
        qs.dma_start(xts[0][:, 0:512], XT[0:P, 0:512])
        for piece in range(3):  # q | k | v slices of the kc=0 weight chunk
            psl = slice(HD * piece, HD * (piece + 1))
            qa.dma_start(wqkv_sb[:, 0, psl], WQKV[0:P, psl])
        for kc in range(1, NKC):
            qx, qw = (qs, qa) if kc % 2 == 0 else (qa, qs)
            ksl = slice(kc * P, (kc + 1) * P)
            qw.dma_start(wqkv_sb[:, kc, :], WQKV[ksl, :])
            qx.dma_start(xts[kc][:, 0:512], XT[ksl, 0:512])
            if kc == 1:
                qs.dma_start(mbig_sb, MBIG.bitcast(MM_DT))
            if kc == 2:
                qs.dma_start(bias_sb, BQK)
                qa.dma_start(ones_sb, ONES.bitcast(MM_DT))
            if kc == 4:
                qs.dma_start(bv_sb, BV)
                qa.dma_start(v_sb[:, :, :, D : D + 1], VONES.bitcast(MM_DT))
        for t in range(1, NT):
            sl_ = slice(512 * t, 512 * (t + 1))
            for kc in range(NKC):
                qx = qs if (kc + t) % 2 == 0 else qa
                qx.dma_start(xts[kc][:, sl_], XT[kc * P : (kc + 1) * P, sl_])
            if t == 1:
                qa.dma_start(wo_sb, WO.rearrange("(a p) c -> p a c", p=P))

        # Single PSUM pool; stages share tag families so the Tile scheduler can
        # pipeline projections, attention and output projection freely.
        # Banks: big 2x2 + pot 2x1 + sm 2x1 = 8.
        psum = ctx.enter_context(tc.tile_pool(name="psum", bufs=2, space="PSUM"))
        drsc = ctx.enter_context(tc.tile_pool(name="drsc", bufs=2, space="DRAM"))
        spool = ctx.enter_context(tc.tile_pool(name="spool", bufs=5))
        npool = ctx.enter_context(tc.tile_pool(name="npool", bufs=2))
        opool = ctx.enter_context(tc.tile_pool(name="opool", bufs=2))

        # ---- stage A: projections for one i-tile ----
        def stage_a(t):
            sl = slice(512 * t, 512 * (t + 1))
            for m in range(2):
                msl = slice(P * m, P * (m + 1))
                pqk = psum.tile([P, 1024], F32, tag="big", bufs=3)
                for kc in range(NKC):
                    nc.tensor.matmul(
                        pqk[:, 0:512],
                        lhsT=wq_c(kc, msl),
                        rhs=xts[kc][:, sl],
                        start=(kc == 0),
                        stop=(kc == NKC - 1),
                    )
                    nc.tensor.matmul(
                        pqk[:, 512:1024],
                        lhsT=wk_c(kc, msl),
                        rhs=xts[kc][:, sl],
                        start=(kc == 0),
                        stop=(kc == NKC - 1),
                    )
                nc.vector.tensor_scalar_add(
                    qt_sb[m][:, sl], pqk[:, 0:512], bias_sb[:, m : m + 1]
                )
                nc.vector.tensor_scalar_add(
                    kt_sb[m][:, sl], pqk[:, 512:1024], bias_sb[:, 2 + m : 3 + m]
                )
            for ic in range(4 * t, 4 * (t + 1)):
                isl = slice(P * ic, P * (ic + 1))
                pv = psum.tile([P, HD], F32, tag="sm", bufs=1)
                for kc in range(NKC):
                    nc.tensor.matmul(
                        pv,
                        lhsT=xts[kc][:, isl],
                        rhs=wv_c(kc),
                        start=(kc == 0),
                        stop=False,
                    )
                nc.tensor.matmul(
                    pv, lhsT=ones_sb, rhs=bv_sb, start=False, stop=True
                )
                nc.vector.tensor_copy(
                    v_sb[:, ic, :, 0:D], pv.rearrange("p (h d) -> p h d", d=D)
                )

        # ---- stage B: attention for one i-tile ----
        def stage_b(t, last_tile):
            sl = slice(512 * t, 512 * (t + 1))
            for l in range(HPC):
                mc, ro = l // 2, 64 * (l % 2)
                qrow = slice(ro, ro + 64)
                pot = psum.tile([D + 1, 512], F32, tag="pot", bufs=1)
                first = [True]

                def av(jc, rhs, w, stop=False):
                    nc.tensor.matmul(
                        pot[:, 512 - w : 512],
                        lhsT=v_sb[:, jc, l, 0 : D + 1],
                        rhs=rhs,
                        start=first[0],
                        stop=stop,
                        skip_group_check=True,
                    )
                    first[0] = False

                # full (unmasked) chunks, exp'd in pairs
                for jcp in range(2 * t):
                    ps = psum.tile([P, 1024], F32, tag="big", bufs=3)
                    for half in (0, 1):
                        jc = 2 * jcp + half
                        nc.tensor.matmul(
                            ps[:, 512 * half : 512 * (half + 1)],
                            lhsT=kt_sb[mc][qrow, P * jc : P * (jc + 1)],
                            rhs=qt_sb[mc][qrow, sl],
                            start=True,
                            stop=True,
                        )
                    ex = spool.tile([P, 1024], MM_DT, tag="ex")
                    nc.scalar.activation(ex, ps, AF.Exp)
                    for half in (0, 1):
                        av(2 * jcp + half, ex[:, 512 * half : 512 * (half + 1)], 512)

                # diagonal chunks k=0..3 (jc = 4t+k), truncated to the valid
                # i-range: width w = 512-128k, covering i in [512t+128k, ...).
                # Element j of the region maps to f = 128k+j; valid iff j >= p,
                # i.e. mask slice mbig[:, 384 : 384+w].
                widths = {0: 512, 1: 384, 2: 256, 3: 128}
                for ka, kb in ((0, 1), (2, 3)):
                    ps = psum.tile([P, 1024], F32, tag="big", bufs=3)
                    for half, k in ((0, ka), (1, kb)):
                        w = widths[k]
                        nc.tensor.matmul(
                            ps[:, 512 * half : 512 * half + w],
                            lhsT=kt_sb[mc][qrow, P * (4 * t + k) : P * (4 * t + k + 1)],
                            rhs=qt_sb[mc][qrow, 512 * (t + 1) - w : 512 * (t + 1)],
                            start=True,
                            stop=True,
                        )
                    wb = widths[kb]
                    ex = spool.tile([P, 1024], MM_DT, tag="ex")
                    nc.scalar.activation(ex[:, 0 : 512 + wb], ps[:, 0 : 512 + wb], AF.Exp)
                    for half, k in ((0, ka), (1, kb)):
                        w = widths[k]
                        exh = ex[:, 512 * half : 512 * half + w]
                        eng = nc.gpsimd if k == 0 else nc.vector
                        eng.tensor_mul(exh, exh, mbig_sb[:, 0:w])
                        av(4 * t + k, exh, w, stop=(ka == 2 and half == 1))
                # Evacuate pot to SBUF right away (frees the PSUM slot), then
                # normalize by the softmax sum (row D). The 64-partition
                # broadcast of the reciprocal goes through a DRAM-bounce DMA
                # (off the PSUM critical path); on the final tile a K=1 PE
                # outer product is used instead — lower latency, and PSUM
                # pressure no longer matters at the tail.
                potc = npool.tile([D + 1, 512], F32, tag="potc")
                nc.vector.tensor_copy(potc, pot)
                rc = npool.tile([1, 512], MM_DT, tag="rc")
                with nc.allow_low_precision(reason="mm-dtype rounding of recip"):
                    nc.vector.reciprocal(rc, potc[D : D + 1, :])
                if last_tile:
                    psb = psum.tile([64, 512], F32, tag="pot", bufs=1)
                    nc.tensor.matmul(
                        psb, lhsT=ones_sb[0:1, 0:64], rhs=rc, start=True, stop=True
                    )
                    nc.vector.tensor_mul(ot_sb[mc][qrow, sl], potc[0:D, :], psb)
                else:
                    rd = drsc.tile([1, 512], F32, tag="rd")
                    qs.dma_start(rd, rc.bitcast(F32))
                    bc = npool.tile([64, 512], F32, tag="bc")
                    qs.dma_start(bc, rd.to_broadcast((64, 512)))
                    nc.vector.tensor_mul(ot_sb[mc][qrow, sl], potc[0:D, :], bc)

        # ---- stage C: output projection for the 4 i-chunks of one i-tile ----
        def stage_c(t, last_tile):
            for ic in range(4 * t, 4 * (t + 1)):
                isl = slice(P * ic, P * (ic + 1))
                ob = opool.tile([P, C], F32, tag="ob")
                for n in (0, 1):
                    po = psum.tile([P, 512], F32, tag="sm", bufs=1)
                    for kc in range(2):
                        nc.tensor.matmul(
                            po,
                            lhsT=ot_sb[kc][:, isl],
                            rhs=wo_sb[:, kc, 512 * n : 512 * (n + 1)],
                            start=(kc == 0),
                            stop=(kc == 1),
                        )
                    if n == 0 and ic % 2 == 0:
                        nc.scalar.copy(ob[:, 0:512], po)
                    else:
                        nc.vector.tensor_copy(ob[:, 512 * n : 512 * (n + 1)], po)
                    if last_tile:  # store halves eagerly to shorten the tail
                        qs.dma_start(
                            OUT[isl, 512 * n : 512 * (n + 1)],
                            ob[:, 512 * n : 512 * (n + 1)],
                        )
                if not last_tile:
                    qs.dma_start(OUT[isl, :], ob)

        # Emission order: all projections first, then attention tiles in order
        # 0,2,3,1 — C(t) of each earlier tile backfills PE during the next
        # tile's ACT-bound stretch, and the final tile (t=1) has a short tail.
        for t in range(NT):
            stage_a(t)
        for ti, t in enumerate((0, 2, 3, 1)):
            stage_b(t, ti == 3)
            stage_c(t, ti == 3)


def _get_program():
    if "nc" not in _CACHE:
        _CACHE["nc"] = _build_program()
    return _CACHE["nc"]


class _Runner:
    """Reusable SPMD executor (adapted from concourse.bass2jax.run_bass_via_pjrt)
    so repeated kernel() calls reuse one compiled executable."""

    def __init__(self, nc):
        import jax
        import concourse.mybir as mb
        from jax.sharding import Mesh, PartitionSpec
        from jax.experimental.shard_map import shard_map
        from concourse import bass2jax

        bass2jax.install_neuronx_cc_hook()
        self.jax = jax
        self.nc = nc
        partition_name = (
            nc.partition_id_tensor.name if nc.partition_id_tensor else None
        )
        in_names, out_names, out_avals, zero_outs = [], [], [], []
        for alloc in nc.m.functions[0].allocations:
            if not isinstance(alloc, mb.MemoryLocationSet):
                continue
            name = alloc.memorylocations[0].name
            if alloc.kind == "ExternalInput":
                if name != partition_name:
                    in_names.append(name)
            elif alloc.kind == "ExternalOutput":
                shape = tuple(alloc.tensor_shape)
                dtype = mb.dt.np(alloc.dtype)
                out_names.append(name)
                out_avals.append(jax.core.ShapedArray(shape, dtype))
                zero_outs.append((shape, dtype))
        self.n_params = len(in_names)
        self.in_names = list(in_names)
        self.out_names = out_names
        self.out_avals = out_avals
        self.zero_outs = zero_outs
        all_in_names = in_names + out_names + (
            [partition_name] if partition_name else []
        )
        donate = tuple(range(self.n_params, self.n_params + len(out_names)))

        def _body(*args):
            operands = list(args)
            if partition_name is not None:
                operands.append(bass2jax.partition_id_tensor())
            outs = bass2jax._bass_exec_p.bind(
                *operands,
                out_avals=tuple(out_avals),
                in_names=tuple(all_in_names),
                out_names=tuple(out_names),
                lowering_input_output_aliases=(),
                sim_require_finite=True,
                sim_require_nnan=True,
                nc=nc,
            )
            return tuple(outs)

        devices = jax.devices()[:N_CORES]
        self.mesh = Mesh(np.asarray(devices), ("core",))
        in_specs = (PartitionSpec("core"),) * (self.n_params + len(out_names))
        out_specs = (PartitionSpec("core"),) * len(out_names)
        self.sharded = jax.jit(
            shard_map(
                _body,
                mesh=self.mesh,
                in_specs=in_specs,
                out_specs=out_specs,
                check_rep=False,
            ),
            donate_argnums=donate,
            keep_unused=True,
        )

    def concat_inputs(self, in_maps):
        return [
            np.concatenate([np.asarray(m[name]) for m in in_maps], axis=0)
            for name in self.in_names
        ]

    def zeros(self):
        return [
            np.zeros((N_CORES * s[0], *s[1:]), d) for s, d in self.zero_outs
        ]

    def run(self, concat_in, zeros):
        out_arrs = self.sharded(*concat_in, *zeros)
        return out_arrs

    def split(self, out_arrs):
        res = []
        for c in range(N_CORES):
            res.append(
                {
                    name: np.asarray(out_arrs[i]).reshape(
                        N_CORES, *self.out_avals[i].shape
                    )[c]
                    for i, name in enumerate(self.out_names)
                }
            )
        return res


def _get_runner():
    if "runner" not in _CACHE:
        _CACHE["runner"] = _Runner(_get_program())
    return _CACHE["runner"]


def _shard_inputs(X, Wq, bq, Wk, bk, Wv, bv, Wo, bo):
    in_maps = []
    for c in range(N_CORES):
        b, hg = divmod(c, HG)
        cols = slice(HD * hg, HD * (hg + 1))
        bqk = np.stack(
            [
                bq[cols][:P] * 0.125,
                bq[cols][P:] * 0.125,
                bk[cols][:P],
                bk[cols][P:],
            ],
            axis=1,
        ).astype(np.float32)
        in_maps.append(
            {
                "XT": np.ascontiguousarray(X[b].T),
                "WQKV": np.concatenate(
                    [Wq[:, cols] * 0.125, Wk[:, cols], Wv[:, cols]], axis=1
                ).astype(np.float32),
                "BQK": bqk,
                "BV": bv[cols].reshape(1, HD).astype(np.float32),
                "WO": np.ascontiguousarray(Wo[cols, :]),
            }
        )
    return in_maps


def kernel(X, Wq, bq, Wk, bk, Wv, bv, Wo, bo):
    X = np.asarray(X, dtype=np.float32)
    Wq, bq = np.asarray(Wq, np.float32), np.asarray(bq, np.float32)
    Wk, bk = np.asarray(Wk, np.float32), np.asarray(bk, np.float32)
    Wv, bv = np.asarray(Wv, np.float32), np.asarray(bv, np.float32)
    Wo, bo = np.asarray(Wo, np.float32), np.asarray(bo, np.float32)

    runner = _get_runner()
    in_maps = _shard_inputs(X, Wq, bq, Wk, bk, Wv, bv, Wo, bo)
    res = runner.split(runner.run(runner.concat_inputs(in_maps), runner.zeros()))

    out = np.empty((B, T, C), dtype=np.float32)
    for b in range(B):
        acc = np.zeros((T, C), dtype=np.float64)
        for hg in range(HG):
            acc += res[HG * b + hg]["OUT"]
        out[b] = (acc + bo.astype(np.float64)).astype(np.float32)
    return out


# revision 78
# speedup vs baseline: 1.0012x; 1.0012x over previous
"""Causal multi-head attention block (B=2, T=2048, C=1024, H=16) on 8 TRN2 cores.

Sharding: tensor-parallel over heads x data-parallel over batch.
Core c handles batch b = c // 4 and head-group hg = c % 4 (4 heads = 256 of
the 1024 channel columns). Each core computes, for its batch and heads:
    QT/KT = (Wslice/8)^T X^T + b/8   (scores pre-scaled by 1/sqrt(D))
    V     = X Wv_slice + bv
    S^T   = K Q^T (causal, streamed in 128x512 chunks), P = exp(S^T)
    O^T   = [V|1]^T P  -> rows 0..63 unnormalized output, row 64 softmax sum
    partial = (O^T normalized)^T @ Wo_rows_slice        [2048, 1024]
Host sums the 4 partials per batch and adds bo.

MM_DT selects the PE datapath: float32r (full-rate fp32 path, ~1.6e-4 rel
per matmul) or float32 (exact, 4x slower rows).
"""

from contextlib import ExitStack

import numpy as np

import concourse.bacc as bacc
import concourse.mybir as mybir
import concourse.tile as tile
from concourse.bass_utils import run_bass_kernel_spmd

B, T, C, H, D = 2, 2048, 1024, 16, 64
N_CORES = 8
HG = 4                  # head-groups (tensor parallel)
HPC = H // HG           # heads per core = 4
HD = HPC * D            # channel slice per core = 256
P = 128                 # partitions
NT = T // 512           # 4 i-tiles of 512
NIC = T // P            # 16 i-chunks of 128
NKC = C // P            # 8 contraction chunks of 128
F32 = mybir.dt.float32
F32R = mybir.dt.float32r
AF = mybir.ActivationFunctionType

MM_DT = F32R            # matmul datapath dtype (F32R fast / F32 exact)

_CACHE: dict = {}


def _build_program():
    nc = bacc.Bacc("TRN2", debug=False)

    XT = nc.dram_tensor("XT", [C, T], MM_DT, kind="ExternalInput").ap()
    WQKV = nc.dram_tensor("WQKV", [C, 3 * HD], MM_DT, kind="ExternalInput").ap()
    BQK = nc.dram_tensor("BQK", [P, 4], F32, kind="ExternalInput").ap()
    BV = nc.dram_tensor("BV", [1, HD], MM_DT, kind="ExternalInput").ap()
    WO = nc.dram_tensor("WO", [HD, C], MM_DT, kind="ExternalInput").ap()
    OUT = nc.dram_tensor("OUT", [T, C], F32, kind="ExternalOutput").ap()

    # Causal chunk mask: for diagonal chunk k (k=0..3), valid iff f >= p + 128k,
    # realized as slices of Mbig[p, x] = (x >= p + 384).
    mb = (np.arange(512)[None, :] >= np.arange(P)[:, None]).astype(np.float32)
    MBIG = nc.inline_tensor(mb, name="mbig").ap()
    ONES = nc.inline_tensor(np.ones((1, P), np.float32), name="ones_c").ap()
    VONES = nc.inline_tensor(np.ones((P, NIC * HPC), np.float32), name="vones_c").ap()

    with tile.TileContext(nc) as tc:
        _trace_kernel(tc, XT, WQKV, BQK, BV, WO, OUT, MBIG, ONES, VONES)
    nc.compile()
    return nc


def _trace_kernel(tc, XT, WQKV, BQK, BV, WO, OUT, MBIG, ONES, VONES):
    nc = tc.nc

    with ExitStack() as ctx:
        consts = ctx.enter_context(tc.tile_pool(name="consts", bufs=1))
        wpool = ctx.enter_context(tc.tile_pool(name="weights", bufs=1))
        xpool = ctx.enter_context(tc.tile_pool(name="xt", bufs=1))
        qkv = ctx.enter_context(tc.tile_pool(name="qkv", bufs=1))

        # Two HWDGE queues: SP (nc.sync) and Activation (nc.scalar). Interleave
        # the big loads across both so stage A is compute- not DMA-paced.
        qs, qa = nc.sync, nc.scalar

        # ---- tiles ----
        mbig_sb = consts.tile([P, 512], MM_DT, name="mbig_sb")
        ones_sb = consts.tile([1, P], MM_DT, name="ones_sb")
        bias_sb = consts.tile([P, 4], F32, name="bias_sb")  # bq m0,m1, bk m0,m1
        bv_sb = consts.tile([1, HD], MM_DT, name="bv_sb")
        wqkv_sb = wpool.tile([P, NKC, 3 * HD], MM_DT, name="wqkv_sb")
        wo_sb = wpool.tile([P, 2, C], MM_DT, name="wo_sb")
        xts = [
            xpool.tile([P, T], MM_DT, name=f"xt{kc}", tag=f"xt{kc}")
            for kc in range(NKC)
        ]
        qt_sb = [qkv.tile([P, T], MM_DT, name=f"qt{m}", tag=f"qt{m}") for m in range(2)]
        kt_sb = [qkv.tile([P, T], MM_DT, name=f"kt{m}", tag=f"kt{m}") for m in range(2)]
        v_sb = qkv.tile([P, NIC, HPC, D + 1], MM_DT, name="v_sb")
        ot_sb = [qkv.tile([P, T], MM_DT, name=f"ot{m}", tag=f"ot{m}") for m in range(2)]

        def wq_c(kc, msl):
            return wqkv_sb[:, kc, msl]

        def wk_c(kc, msl):
            return wqkv_sb[:, kc, slice(HD + msl.start, HD + msl.stop)]

        def wv_c(kc):
            return wqkv_sb[:, kc, 2 * HD : 3 * HD]

        # Preload the ACT Exp table while the first DMAs stream (the table
        # load costs ~1.3us and would otherwise land on the first real exp).
        scx = consts.tile([1, 1], F32, name="scx")
        nc.vector.memset(scx, 0.0)
        scy = consts.tile([1, 1], F32, name="scy")
        nc.scalar.activation(scy, scx, AF.Exp)

        # ---- loads ----
        # Weights and the t=0 column-block of X^T stream first (kc-ordered,
        # alternating across both HWDGE queues) so t=0 projections complete
        # after ~5MB; remaining X^T column-blocks stream per i-tile behind
        # them and the whole pipeline becomes PE-paced after ~15us.
        qs.dma_start(xts[0][:, 0:512], XT[0:P, 0:512])
        for piece in range(3):  # q | k | v slices of the kc=0 weight chunk
            psl = slice(HD * piece, HD * (piece + 1))
            qa.dma_start(wqkv_sb[:, 0, psl], WQKV[0:P, psl])
        for kc in range(1, NKC):
            qx, qw = (qs, qa) if kc % 2 == 0 else (qa, qs)
            ksl = slice(kc * P, (kc + 1) * P)
            qw.dma_start(wqkv_sb[:, kc, :], WQKV[ksl, :])
            qx.dma_start(xts[kc][:, 0:512], XT[ksl, 0:512])
            if kc == 1:
                qs.dma_start(mbig_sb, MBIG.bitcast(MM_DT))
            if kc == 2:
                qs.dma_start(bias_sb, BQK)
                qa.dma_start(ones_sb, ONES.bitcast(MM_DT))
            if kc == 4:
                qs.dma_start(bv_sb, BV)
                qa.dma_start(v_sb[:, :, :, D : D + 1], VONES.bitcast(MM_DT))
        for t in range(1, NT):
            sl_ = slice(512 * t, 512 * (t + 1))
            for kc in range(NKC):
                qx = qs if (kc + t) % 2 == 0 else qa
                qx.dma_start(xts[kc][:, sl_], XT[kc * P : (kc + 1) * P, sl_])
            if t == 1:
                qa.dma_start(wo_sb, WO.rearrange("(a p) c -> p a c", p=P))

        # Single PSUM pool; stages share tag families so the Tile scheduler can
        # pipeline projections, attention and output projection freely.
        # Banks: big 2x2 + pot 2x1 + sm 2x1 = 8.
        psum = ctx.enter_context(tc.tile_pool(name="psum", bufs=2, space="PSUM"))
        drsc = ctx.enter_context(tc.tile_pool(name="drsc", bufs=4, space="DRAM"))
        spool = ctx.enter_context(tc.tile_pool(name="spool", bufs=5))
        npool = ctx.enter_context(tc.tile_pool(name="npool", bufs=2))
        opool = ctx.enter_context(tc.tile_pool(name="opool", bufs=3))

        # ---- stage A: projections for one i-tile ----
        def stage_a(t):
            sl = slice(512 * t, 512 * (t + 1))
            for m in range(2):
                msl = slice(P * m, P * (m + 1))
                pqk = psum.tile([P, 1024], F32, tag="big", bufs=3)
                for kc in range(NKC):
                    nc.tensor.matmul(
                        pqk[:, 0:512],
                        lhsT=wq_c(kc, msl),
                        rhs=xts[kc][:, sl],
                        start=(kc == 0),
                        stop=(kc == NKC - 1),
                    )
                    nc.tensor.matmul(
                        pqk[:, 512:1024],
                        lhsT=wk_c(kc, msl),
                        rhs=xts[kc][:, sl],
                        start=(kc == 0),
                        stop=(kc == NKC - 1),
                    )
                nc.vector.tensor_scalar_add(
                    qt_sb[m][:, sl], pqk[:, 0:512], bias_sb[:, m : m + 1]
                )
                nc.vector.tensor_scalar_add(
                    kt_sb[m][:, sl], pqk[:, 512:1024], bias_sb[:, 2 + m : 3 + m]
                )
            for ic in range(4 * t, 4 * (t + 1)):
                isl = slice(P * ic, P * (ic + 1))
                pv = psum.tile([P, HD], F32, tag="sm", bufs=1)
                for kc in range(NKC):
                    nc.tensor.matmul(
                        pv,
                        lhsT=xts[kc][:, isl],
                        rhs=wv_c(kc),
                        start=(kc == 0),
                        stop=False,
                    )
                nc.tensor.matmul(
                    pv, lhsT=ones_sb, rhs=bv_sb, start=False, stop=True
                )
                nc.vector.tensor_copy(
                    v_sb[:, ic, :, 0:D], pv.rearrange("p (h d) -> p h d", d=D)
                )

        # ---- stage B: attention for one i-tile ----
        def stage_b(t, last_tile):
            sl = slice(512 * t, 512 * (t + 1))
            for l in range(HPC):
                mc, ro = l // 2, 64 * (l % 2)
                qrow = slice(ro, ro + 64)
                pot = psum.tile([D + 1, 512], F32, tag="pot", bufs=1)
                first = [True]

                def av(jc, rhs, w, stop=False):
                    nc.tensor.matmul(
                        pot[:, 512 - w : 512],
                        lhsT=v_sb[:, jc, l, 0 : D + 1],
                        rhs=rhs,
                        start=first[0],
                        stop=stop,
                        skip_group_check=True,
                    )
                    first[0] = False

                # full (unmasked) chunks, exp'd in pairs
                for jcp in range(2 * t):
                    ps = psum.tile([P, 1024], F32, tag="big", bufs=3)
                    for half in (0, 1):
                        jc = 2 * jcp + half
                        nc.tensor.matmul(
                            ps[:, 512 * half : 512 * (half + 1)],
                            lhsT=kt_sb[mc][qrow, P * jc : P * (jc + 1)],
                            rhs=qt_sb[mc][qrow, sl],
                            start=True,
                            stop=True,
                        )
                    ex = spool.tile([P, 1024], MM_DT, tag="ex")
                    nc.scalar.activation(ex, ps, AF.Exp)
                    for half in (0, 1):
                        av(2 * jcp + half, ex[:, 512 * half : 512 * (half + 1)], 512)

                # diagonal chunks k=0..3 (jc = 4t+k), truncated to the valid
                # i-range: width w = 512-128k, covering i in [512t+128k, ...).
                # Element j of the region maps to f = 128k+j; valid iff j >= p,
                # i.e. mask slice mbig[:, 384 : 384+w].
                widths = {0: 512, 1: 384, 2: 256, 3: 128}
                for ka, kb in ((0, 1), (2, 3)):
                    ps = psum.tile([P, 1024], F32, tag="big", bufs=3)
                    for half, k in ((0, ka), (1, kb)):
                        w = widths[k]
                        nc.tensor.matmul(
                            ps[:, 512 * half : 512 * half + w],
                            lhsT=kt_sb[mc][qrow, P * (4 * t + k) : P * (4 * t + k + 1)],
                            rhs=qt_sb[mc][qrow, 512 * (t + 1) - w : 512 * (t + 1)],
                            start=True,
                            stop=True,
                        )
                    wb = widths[kb]
                    ex = spool.tile([P, 1024], MM_DT, tag="ex")
                    nc.scalar.activation(ex[:, 0 : 512 + wb], ps[:, 0 : 512 + wb], AF.Exp)
                    for half, k in ((0, ka), (1, kb)):
                        w = widths[k]
                        exh = ex[:, 512 * half : 512 * half + w]
                        eng = nc.gpsimd if k == 0 else nc.vector
                        eng.tensor_mul(exh, exh, mbig_sb[:, 0:w])
                        av(4 * t + k, exh, w, stop=(ka == 2 and half == 1))
                # Evacuate pot to SBUF right away (frees the PSUM slot), then
                # normalize by the softmax sum (row D). The 64-partition
                # broadcast of the reciprocal goes through a DRAM-bounce DMA
                # (off the PSUM critical path); on the final tile a K=1 PE
                # outer product is used instead — lower latency, and PSUM
                # pressure no longer matters at the tail.
                potc = npool.tile([D + 1, 512], F32, tag="potc")
                nc.vector.tensor_copy(potc, pot)
                rc = npool.tile([1, 512], MM_DT, tag="rc")
                with nc.allow_low_precision(reason="mm-dtype rounding of recip"):
                    nc.vector.reciprocal(rc, potc[D : D + 1, :])
                if last_tile:
                    psb = psum.tile([64, 512], F32, tag="pot", bufs=1)
                    nc.tensor.matmul(
                        psb, lhsT=ones_sb[0:1, 0:64], rhs=rc, start=True, stop=True
                    )
                    nc.vector.tensor_mul(ot_sb[mc][qrow, sl], potc[0:D, :], psb)
                else:
                    rd = drsc.tile([1, 512], F32, tag="rd")
                    qs.dma_start(rd, rc.bitcast(F32))
                    bc = npool.tile([64, 512], F32, tag="bc")
                    qs.dma_start(bc, rd.to_broadcast((64, 512)))
                    nc.vector.tensor_mul(ot_sb[mc][qrow, sl], potc[0:D, :], bc)

        # ---- stage C: output projection for the 4 i-chunks of one i-tile ----
        def stage_c(t, last_tile):
            for ic in range(4 * t, 4 * (t + 1)):
                isl = slice(P * ic, P * (ic + 1))
                ob = opool.tile([P, C], F32, tag="ob")
                for n in (0, 1):
                    po = psum.tile([P, 512], F32, tag="sm", bufs=1)
                    for kc in range(2):
                        nc.tensor.matmul(
                            po,
                            lhsT=ot_sb[kc][:, isl],
                            rhs=wo_sb[:, kc, 512 * n : 512 * (n + 1)],
                            start=(kc == 0),
                            stop=(kc == 1),
                        )
                    if n == 0 and ic % 2 == 0:
                        nc.scalar.copy(ob[:, 0:512], po)
                    else:
                        nc.vector.tensor_copy(ob[:, 512 * n : 512 * (n + 1)], po)
                    if last_tile:  # store halves eagerly to shorten the tail
                        qs.dma_start(
                            OUT[isl, 512 * n : 512 * (n + 1)],
                            ob[:, 512 * n : 512 * (n + 1)],
                        )
                if not last_tile:
                    qs.dma_start(OUT[isl, :], ob)

        # Emission order: all projections first, then attention tiles in order
        # 0,2,3,1 — C(t) of each earlier tile backfills PE during the next
        # tile's ACT-bound stretch, and the final tile (t=1) has a short tail.
        for t in range(NT):
            stage_a(t)
        for ti, t in enumerate((0, 2, 3, 1)):
            stage_b(t, ti == 3)
            stage_c(t, ti == 3)


def _get_program():
    if "nc" not in _CACHE:
        _CACHE["nc"] = _build_program()
    return _CACHE["nc"]


class _Runner:
    """Reusable SPMD executor (adapted from concourse.bass2jax.run_bass_via_pjrt)
    so repeated kernel() calls reuse one compiled executable."""

    def __init__(self, nc):
        import jax
        import concourse.mybir as mb
        from jax.sharding import Mesh, PartitionSpec
        from jax.experimental.shard_map import shard_map
        from concourse import bass2jax

        bass2jax.install_neuronx_cc_hook()
        self.jax = jax
        self.nc = nc
        partition_name = (
            nc.partition_id_tensor.name if nc.partition_id_tensor else None
        )
        in_names, out_names, out_avals, zero_outs = [], [], [], []
        for alloc in nc.m.functions[0].allocations:
            if not isinstance(alloc, mb.MemoryLocationSet):
                continue
            name = alloc.memorylocations[0].name
            if alloc.kind == "ExternalInput":
                if name != partition_name:
                    in_names.append(name)
            elif alloc.kind == "ExternalOutput":
                shape = tuple(alloc.tensor_shape)
                dtype = mb.dt.np(alloc.dtype)
                out_names.append(name)
                out_avals.append(jax.core.ShapedArray(shape, dtype))
                zero_outs.append((shape, dtype))
        self.n_params = len(in_names)
        self.in_names = list(in_names)
        self.out_names = out_names
        self.out_avals = out_avals
        self.zero_outs = zero_outs
        all_in_names = in_names + out_names + (
            [partition_name] if partition_name else []
        )
        donate = tuple(range(self.n_params, self.n_params + len(out_names)))

        def _body(*args):
            operands = list(args)
            if partition_name is not None:
                operands.append(bass2jax.partition_id_tensor())
            outs = bass2jax._bass_exec_p.bind(
                *operands,
                out_avals=tuple(out_avals),
                in_names=tuple(all_in_names),
                out_names=tuple(out_names),
                lowering_input_output_aliases=(),
                sim_require_finite=True,
                sim_require_nnan=True,
                nc=nc,
            )
            return tuple(outs)

        devices = jax.devices()[:N_CORES]
        self.mesh = Mesh(np.asarray(devices), ("core",))
        in_specs = (PartitionSpec("core"),) * (self.n_params + len(out_names))
        out_specs = (PartitionSpec("core"),) * len(out_names)
        self.sharded = jax.jit(
            shard_map(
                _body,
                mesh=self.mesh,
                in_specs=in_specs,
                out_specs=out_specs,
                check_rep=False,
            ),
            donate_argnums=donate,
            keep_unused=True,
        )

    def concat_inputs(self, in_maps):
        return [
            np.concatenate([np.asarray(m[name]) for m in in_maps], axis=0)
            for name in self.in_names
        ]

    def zeros(self):
        return [
            np.zeros((N_CORES * s[0], *s[1:]), d) for s, d in self.zero_outs
        ]

    def run(self, concat_in, zeros):
        out_arrs = self.sharded(*concat_in, *zeros)
        return out_arrs

    def split(self, out_arrs):
        res = []
        for c in range(N_CORES):
            res.append(
                {
                    name: np.asarray(out_arrs[i]).reshape(
                        N_CORES, *self.out_avals[i].shape
                    )[c]
                    for i, name in enumerate(self.out_names)
                }
            )
        return res


def _get_runner():
    if "runner" not in _CACHE:
        _CACHE["runner"] = _Runner(_get_program())
    return _CACHE["runner"]


def _shard_inputs(X, Wq, bq, Wk, bk, Wv, bv, Wo, bo):
    in_maps = []
    for c in range(N_CORES):
        b, hg = divmod(c, HG)
        cols = slice(HD * hg, HD * (hg + 1))
        bqk = np.stack(
            [
                bq[cols][:P] * 0.125,
                bq[cols][P:] * 0.125,
                bk[cols][:P],
                bk[cols][P:],
            ],
            axis=1,
        ).astype(np.float32)
        in_maps.append(
            {
                "XT": np.ascontiguousarray(X[b].T),
                "WQKV": np.concatenate(
                    [Wq[:, cols] * 0.125, Wk[:, cols], Wv[:, cols]], axis=1
                ).astype(np.float32),
                "BQK": bqk,
                "BV": bv[cols].reshape(1, HD).astype(np.float32),
                "WO": np.ascontiguousarray(Wo[cols, :]),
            }
        )
    return in_maps


def kernel(X, Wq, bq, Wk, bk, Wv, bv, Wo, bo):
    X = np.asarray(X, dtype=np.float32)
    Wq, bq = np.asarray(Wq, np.float32), np.asarray(bq, np.float32)
    Wk, bk = np.asarray(Wk, np.float32), np.asarray(bk, np.float32)
    Wv, bv = np.asarray(Wv, np.float32), np.asarray(bv, np.float32)
    Wo, bo = np.asarray(Wo, np.float32), np.asarray(bo, np.float32)

    runner = _get_runner()
    in_maps = _shard_inputs(X, Wq, bq, Wk, bk, Wv, bv, Wo, bo)
    res = runner.split(runner.run(runner.concat_inputs(in_maps), runner.zeros()))

    out = np.empty((B, T, C), dtype=np.float32)
    for b in range(B):
        acc = np.zeros((T, C), dtype=np.float64)
        for hg in range(HG):
            acc += res[HG * b + hg]["OUT"]
        out[b] = (acc + bo.astype(np.float64)).astype(np.float32)
    return out
